# revision 6
# baseline (speedup 1.0000x reference)
"""Trainium2 Bass kernel for nn_Net_274877907022.

Math (see reference):
    AGE_E = lintrans(AGE_inx @ W_age.T + b_age)        row-wise minmax + L2
    h     = tanh(t @ W_prjT.T + b_prjT)                 [N, 256]
    E     = tanh(AGE_E[Endx] @ W_prjL.T + b_prjL)       [M, 256]
    o_c   = h @ (E * outW).T + outb                     [N, M]
    o_f   = h @ W_fc3.T + b_fc3                         [N, 50]

Key optimization: lintrans is purely row-wise, so it is computed only on the
4096 gathered rows AGE_inx[Endx] (4.8 GFLOP) instead of all 20000 rows
(23.6 GFLOP).

Sharding over 8 cores:
  - E-branch sharded by Endx: each core gathers 512 rows, computes its
    G^T = (E*outW).T shard [256, 512], then AllGather -> full G^T [256, 4096].
  - h / o_c / o_f data-parallel over N: each core handles 1024 rows of t.

All compute chains are evaluated in "features on partitions" orientation;
weights are passed pre-transposed from the host (layout prep only), the
gathered AGE rows and z_norm are transposed on-device via the PE.
"""

import numpy as np
import sys

for p in ("/opt/trn_rl_repo",):
    if p not in sys.path:
        sys.path.insert(0, p)

import concourse.bass as bass
import concourse.mybir as mybir
import concourse.tile as tile
from concourse import bass_utils
from concourse.masks import make_identity

F32 = mybir.dt.float32
I32 = mybir.dt.int32

NCORES = 8
N, NODES, DIN, HID, M, OUT_FE = 8192, 20000, 768, 256, 4096, 50
NI = N // NCORES          # 1024 t-rows per core
R = M // NCORES           # 512 gathered rows per core
P = 128
DTILES = DIN // P         # 6
JT = HID // P             # 2
RT = R // P               # 4
IT = NI // P              # 8
MCH = M // 512            # 8 chunks of 512 columns


def build_program(tc: tile.TileContext, io: dict):
    """Emit the kernel IR. `io` maps logical names to DRAM APs."""
    nc = tc.nc

    tT = io["tT"]            # [768, 1024]
    AGE = io["AGE"]          # [20000, 768]
    endx = io["endx"]        # [512] int32
    W_ageT = io["W_ageT"]    # [768, 768]
    b_age = io["b_age"]      # [768]
    W_prjTT = io["W_prjTT"]  # [768, 256]
    b_prjT = io["b_prjT"]    # [256]
    W_prjLT = io["W_prjLT"]  # [768, 256]
    b_prjL = io["b_prjL"]    # [256]
    outWT = io["outWT"]      # [256, 512]   this core's shard of outW, transposed
    outb = io["outb"]        # [4096]
    W_fc3T = io["W_fc3T"]    # [256, 50]
    b_fc3 = io["b_fc3"]      # [50]
    o_c = io["o_c"]          # [1024, 4096] out
    o_f = io["o_f"]          # [1024, 50]   out

    from contextlib import ExitStack
    ctx = ExitStack()
    persist = ctx.enter_context(tc.tile_pool(name="persist", bufs=1))
    arows = ctx.enter_context(tc.tile_pool(name="arows", bufs=2))
    zpool = ctx.enter_context(tc.tile_pool(name="zpool", bufs=2))
    sqpool = ctx.enter_context(tc.tile_pool(name="sqpool", bufs=2))
    stats = ctx.enter_context(tc.tile_pool(name="stats", bufs=8))
    ocstage = ctx.enter_context(tc.tile_pool(name="ocstage", bufs=2))
    psum_t = ctx.enter_context(tc.tile_pool(name="psum_t", bufs=2, space="PSUM"))
    psum_mm = ctx.enter_context(tc.tile_pool(name="psum_mm", bufs=2, space="PSUM"))
    psum_oc = ctx.enter_context(tc.tile_pool(name="psum_oc", bufs=3, space="PSUM"))
    dram = ctx.enter_context(tc.tile_pool(name="dram", bufs=1, space="DRAM"))

    # ---------------- constants / weights into SBUF ----------------
    identity = persist.tile([P, P], F32, tag="identity")
    make_identity(nc, identity)
    ones_col = persist.tile([1, P], F32, tag="ones_col")
    nc.vector.memset(ones_col[:], 1.0)

    b_age_row = persist.tile([1, DIN], F32, tag="b_age_row")
    nc.sync.dma_start(b_age_row[:], b_age[None, :])
    outb_row = persist.tile([1, M], F32, tag="outb_row")
    nc.sync.dma_start(outb_row[:], outb[None, :])
    bfc3_row = persist.tile([1, OUT_FE], F32, tag="bfc3_row")
    nc.sync.dma_start(bfc3_row[:], b_fc3[None, :])

    b_prjT_p = []
    b_prjL_p = []
    for j in range(JT):
        bt = persist.tile([P, 1], F32, tag=f"b_prjT_{j}", name=f"b_prjT_{j}")
        nc.sync.dma_start(bt[:], b_prjT[j * P:(j + 1) * P][:, None])
        b_prjT_p.append(bt)
        bl = persist.tile([P, 1], F32, tag=f"b_prjL_{j}", name=f"b_prjL_{j}")
        nc.sync.dma_start(bl[:], b_prjL[j * P:(j + 1) * P][:, None])
        b_prjL_p.append(bl)

    # indices for the gather
    idx_tiles = []
    for rt in range(RT):
        it_ = persist.tile([P, 1], I32, tag=f"idx_{rt}", name=f"idx_{rt}")
        nc.sync.dma_start(it_[:], endx[rt * P:(rt + 1) * P][:, None])
        idx_tiles.append(it_)

    W_ageT_sb = []
    for d in range(DTILES):
        w = persist.tile([P, DIN], F32, tag=f"W_ageT_{d}", name=f"W_ageT_{d}")
        nc.sync.dma_start(w[:], W_ageT[d * P:(d + 1) * P, :])
        W_ageT_sb.append(w)
    W_prjLT_sb = []
    W_prjTT_sb = []
    for d in range(DTILES):
        w = persist.tile([P, HID], F32, tag=f"W_prjLT_{d}", name=f"W_prjLT_{d}")
        nc.sync.dma_start(w[:], W_prjLT[d * P:(d + 1) * P, :])
        W_prjLT_sb.append(w)
        w2 = persist.tile([P, HID], F32, tag=f"W_prjTT_{d}", name=f"W_prjTT_{d}")
        nc.sync.dma_start(w2[:], W_prjTT[d * P:(d + 1) * P, :])
        W_prjTT_sb.append(w2)
    outWT_sb = []
    W_fc3T_sb = []
    for j in range(JT):
        w = persist.tile([P, R], F32, tag=f"outWT_{j}", name=f"outWT_{j}")
        nc.sync.dma_start(w[:], outWT[j * P:(j + 1) * P, :])
        outWT_sb.append(w)
        w2 = persist.tile([P, OUT_FE], F32, tag=f"W_fc3T_{j}", name=f"W_fc3T_{j}")
        nc.sync.dma_start(w2[:], W_fc3T[j * P:(j + 1) * P, :])
        W_fc3T_sb.append(w2)

    tT_sb = []
    for d in range(DTILES):
        w = persist.tile([P, NI], F32, tag=f"tT_{d}", name=f"tT_{d}")
        nc.sync.dma_start(w[:], tT[d * P:(d + 1) * P, :])
        tT_sb.append(w)

    # ---------------- E branch ----------------
    # gather rows and transpose them: arT[d] holds AGE_rows^T [128(d), 512(r)]
    arT = []
    for d in range(DTILES):
        a = persist.tile([P, R], F32, tag=f"arT_{d}", name=f"arT_{d}")
        arT.append(a)
    for rt in range(RT):
        ar = arows.tile([P, DIN], F32, tag="ar", name=f"ar_{rt}")
        nc.gpsimd.indirect_dma_start(
            out=ar[:],
            out_offset=None,
            in_=AGE[:],
            in_offset=bass.IndirectOffsetOnAxis(ap=idx_tiles[rt][:, :1], axis=0),
        )
        for d in range(DTILES):
            pt = psum_t.tile([P, P], F32, tag="pt", name=f"ptA_{rt}_{d}")
            nc.tensor.transpose(pt[:], ar[:, d * P:(d + 1) * P], identity[:])
            nc.any.tensor_copy(arT[d][:, rt * P:(rt + 1) * P], pt[:])

    # z = AGE_rows @ W_age^T + b_age (natural layout), then lintrans, then
    # transpose z_norm into znT[d] [128(d), 512(r)]
    znT = []
    for d in range(DTILES):
        z = persist.tile([P, R], F32, tag=f"znT_{d}", name=f"znT_{d}")
        znT.append(z)

    for rt in range(RT):
        z_sb = zpool.tile([P, DIN], F32, tag="z_sb", name=f"z_{rt}")
        for ch in range(2):  # two 384-wide output chunks
            zp = psum_mm.tile([P, 512], F32, tag="mm", name=f"zp_{rt}_{ch}")
            zps = zp[:, :384]
            nc.tensor.matmul(zps, ones_col[:], b_age_row[:, ch * 384:(ch + 1) * 384],
                             start=True, stop=False)
            for d in range(DTILES):
                nc.tensor.matmul(
                    zps,
                    arT[d][:, rt * P:(rt + 1) * P],
                    W_ageT_sb[d][:, ch * 384:(ch + 1) * 384],
                    start=False, stop=(d == DTILES - 1),
                )
            nc.any.tensor_copy(z_sb[:, ch * 384:(ch + 1) * 384], zps)

        zmin = stats.tile([P, 1], F32, tag="zmin", name=f"zmin_{rt}")
        zmax = stats.tile([P, 1], F32, tag="zmax", name=f"zmax_{rt}")
        rng = stats.tile([P, 1], F32, tag="rng", name=f"rng_{rt}")
        rsc = stats.tile([P, 1], F32, tag="rsc", name=f"rsc_{rt}")
        ssq = stats.tile([P, 1], F32, tag="ssq", name=f"ssq_{rt}")
        nrm = stats.tile([P, 1], F32, tag="nrm", name=f"nrm_{rt}")
        rnm = stats.tile([P, 1], F32, tag="rnm", name=f"rnm_{rt}")
        nc.vector.tensor_reduce(zmin[:], z_sb[:], axis=mybir.AxisListType.X,
                                op=mybir.AluOpType.min)
        nc.vector.tensor_reduce(zmax[:], z_sb[:], axis=mybir.AxisListType.X,
                                op=mybir.AluOpType.max)
        nc.vector.tensor_tensor(rng[:], zmax[:], zmin[:],
                                op=mybir.AluOpType.subtract)
        nc.vector.reciprocal(rsc[:], rng[:])
        # z01 = (z - zmin) * rsc     (in place)
        nc.vector.tensor_scalar(z_sb[:], z_sb[:], zmin[:], rsc[:],
                                mybir.AluOpType.subtract, mybir.AluOpType.mult)
        # sum of squares -> ssq
        sq = sqpool.tile([P, DIN], F32, tag="sq", name=f"sq_{rt}")
        nc.scalar.activation(sq[:], z_sb[:], mybir.ActivationFunctionType.Square,
                             accum_out=ssq[:])
        nc.scalar.sqrt(nrm[:], ssq[:])
        nc.vector.tensor_scalar_max(nrm[:], nrm[:], 1e-12)
        nc.vector.reciprocal(rnm[:], nrm[:])
        nc.vector.tensor_scalar_mul(z_sb[:], z_sb[:], rnm[:])
        for d in range(DTILES):
            pt = psum_t.tile([P, P], F32, tag="pt", name=f"ptZ_{rt}_{d}")
            nc.tensor.transpose(pt[:], z_sb[:, d * P:(d + 1) * P], identity[:])
            nc.any.tensor_copy(znT[d][:, rt * P:(rt + 1) * P], pt[:])

    # E^T = tanh(W_prjL @ z_norm^T + b_prjL), G^T = E^T * outW^T
    ag_in = dram.tile([HID, R], F32, name="ag_in")
    for j in range(JT):
        ep = psum_mm.tile([P, 512], F32, tag="mm", name=f"ep_{j}")
        for d in range(DTILES):
            nc.tensor.matmul(ep[:], W_prjLT_sb[d][:, j * P:(j + 1) * P], znT[d][:],
                             start=(d == 0), stop=(d == DTILES - 1))
        eT = persist.tile([P, R], F32, tag=f"eT_{j}", name=f"eT_{j}")
        nc.scalar.activation(eT[:], ep[:], mybir.ActivationFunctionType.Tanh,
                             bias=b_prjL_p[j][:, :1], scale=1.0)
        gT = persist.tile([P, R], F32, tag=f"gT_{j}", name=f"gT_{j}")
        nc.vector.tensor_tensor(gT[:], eT[:], outWT_sb[j][:],
                                op=mybir.AluOpType.mult)
        nc.sync.dma_start(ag_in[j * P:(j + 1) * P, :], gT[:])

    # AllGather the G^T shards: [256,512] per core -> [2048,512]
    ag_out = dram.tile([NCORES * HID, R], F32, name="ag_out", addr_space="Shared")
    nc.gpsimd.collective_compute(
        "AllGather",
        mybir.AluOpType.bypass,
        replica_groups=[list(range(NCORES))],
        ins=[ag_in.opt()],
        outs=[ag_out.opt()],
    )

    # ---------------- h branch (independent of E branch) ----------------
    hT = []
    for j in range(JT):
        h = persist.tile([P, NI], F32, tag=f"hT_{j}", name=f"hT_{j}")
        hT.append(h)
    for j in range(JT):
        for ih in range(NI // 512):
            hp = psum_mm.tile([P, 512], F32, tag="mm", name=f"hp_{j}_{ih}")
            for d in range(DTILES):
                nc.tensor.matmul(hp[:], W_prjTT_sb[d][:, j * P:(j + 1) * P],
                                 tT_sb[d][:, ih * 512:(ih + 1) * 512],
                                 start=(d == 0), stop=(d == DTILES - 1))
            nc.scalar.activation(hT[j][:, ih * 512:(ih + 1) * 512], hp[:],
                                 mybir.ActivationFunctionType.Tanh,
                                 bias=b_prjT_p[j][:, :1], scale=1.0)

    # ---------------- o_f = h @ W_fc3^T + b_fc3 ----------------
    of_all = persist.tile([P, IT, OUT_FE], F32, tag="of_all")
    for it in range(IT):
        fp = psum_mm.tile([P, 512], F32, tag="mm", name=f"fp_{it}")
        fps = fp[:, :OUT_FE]
        nc.tensor.matmul(fps, ones_col[:], bfc3_row[:], start=True, stop=False)
        for j in range(JT):
            nc.tensor.matmul(fps, hT[j][:, it * P:(it + 1) * P], W_fc3T_sb[j][:],
                             start=False, stop=(j == JT - 1))
        nc.any.tensor_copy(of_all[:, it, :], fps)
    nc.sync.dma_start(o_f.rearrange("(t p) f -> p t f", p=P), of_all[:])

    # read back the gathered G^T chunks
    GT = [[None] * JT for _ in range(NCORES)]
    for c in range(NCORES):
        for j in range(JT):
            g = persist.tile([P, R], F32, tag=f"GT_{c}_{j}", name=f"GT_{c}_{j}")
            nc.sync.dma_start(g[:], ag_out[(c * JT + j) * P:(c * JT + j + 1) * P, :])
            GT[c][j] = g

    # ---------------- o_c = h @ G^T + outb ----------------
    for it in range(IT):
        for half in range(2):
            stage = ocstage.tile([P, 2048], F32, tag="ocs", name=f"ocs_{it}_{half}")
            for mc in range(4):
                c = half * 4 + mc
                op = psum_oc.tile([P, 512], F32, tag="oc", name=f"op_{it}_{c}")
                nc.tensor.matmul(op[:], ones_col[:],
                                 outb_row[:, c * 512:(c + 1) * 512],
                                 start=True, stop=False)
                for j in range(JT):
                    nc.tensor.matmul(op[:], hT[j][:, it * P:(it + 1) * P],
                                     GT[c][j][:], start=False, stop=(j == JT - 1))
                nc.any.tensor_copy(stage[:, mc * 512:(mc + 1) * 512], op[:])
            nc.sync.dma_start(
                o_c[it * P:(it + 1) * P, half * 2048:(half + 1) * 2048], stage[:])

    ctx.close()


def legalize_sync(nc: bass.Bass):
    """The container's walrus accepts at most one sync wait and one sync
    update per instruction. Split extras onto adjacent NoOps (same engine,
    in-order queue => identical semantics)."""
    cnt = 0
    for func in nc.m.functions:
        for bb in func.blocks:
            new_insts = []
            for inst in bb.instructions:
                si = getattr(inst, "sync_info", None)
                waits = list(si.on_wait) if si is not None and si.on_wait else []
                if si is not None and len(waits) > 1:
                    for w in waits[:-1]:
                        cnt += 1
                        new_insts.append(mybir.InstNoOp(
                            name=f"{inst.name}_sw{cnt}",
                            sync_info=mybir.SyncInfo(on_wait=[w], on_update=[]),
                            bass_nofuse=True,
                            engine=inst.engine,
                        ))
                    si.on_wait = [waits[-1]]
                new_insts.append(inst)
                upds = list(si.on_update) if si is not None and si.on_update else []
                if si is not None and len(upds) > 1:
                    si.on_update = [upds[0]]
                    for u in upds[1:]:
                        cnt += 1
                        new_insts.append(mybir.InstNoOp(
                            name=f"{inst.name}_su{cnt}",
                            sync_info=mybir.SyncInfo(on_wait=[], on_update=[u]),
                            bass_nofuse=True,
                            engine=inst.engine,
                        ))
            bb.instructions[:] = new_insts
    return cnt


def build_bass() -> bass.Bass:
    nc = bass.Bass(trn_type="TRN2", num_devices=NCORES)
    io = {}
    io["tT"] = nc.dram_tensor("tT", [DIN, NI], F32, kind="ExternalInput").ap()
    io["AGE"] = nc.dram_tensor("AGE", [NODES, DIN], F32, kind="ExternalInput").ap()
    io["endx"] = nc.dram_tensor("endx", [R], I32, kind="ExternalInput").ap()
    io["W_ageT"] = nc.dram_tensor("W_ageT", [DIN, DIN], F32, kind="ExternalInput").ap()
    io["b_age"] = nc.dram_tensor("b_age", [DIN], F32, kind="ExternalInput").ap()
    io["W_prjTT"] = nc.dram_tensor("W_prjTT", [DIN, HID], F32, kind="ExternalInput").ap()
    io["b_prjT"] = nc.dram_tensor("b_prjT", [HID], F32, kind="ExternalInput").ap()
    io["W_prjLT"] = nc.dram_tensor("W_prjLT", [DIN, HID], F32, kind="ExternalInput").ap()
    io["b_prjL"] = nc.dram_tensor("b_prjL", [HID], F32, kind="ExternalInput").ap()
    io["outWT"] = nc.dram_tensor("outWT", [HID, R], F32, kind="ExternalInput").ap()
    io["outb"] = nc.dram_tensor("outb", [M], F32, kind="ExternalInput").ap()
    io["W_fc3T"] = nc.dram_tensor("W_fc3T", [HID, OUT_FE], F32, kind="ExternalInput").ap()
    io["b_fc3"] = nc.dram_tensor("b_fc3", [OUT_FE], F32, kind="ExternalInput").ap()
    io["o_c"] = nc.dram_tensor("o_c", [NI, M], F32, kind="ExternalOutput").ap()
    io["o_f"] = nc.dram_tensor("o_f", [NI, OUT_FE], F32, kind="ExternalOutput").ap()
    with tile.TileContext(nc) as tc:
        build_program(tc, io)
    legalize_sync(nc)
    return nc


def make_in_maps(t, AGE_inx, Endx, W_age, b_age, W_prjT, b_prjT, W_prjL,
                 b_prjL, outW, outb, W_fc3, b_fc3):
    f = lambda a: np.ascontiguousarray(np.asarray(a, np.float32))
    tT = f(np.asarray(t, np.float32).T)          # [768, 8192]
    W_ageT = f(np.asarray(W_age).T)
    W_prjTT = f(np.asarray(W_prjT).T)
    W_prjLT = f(np.asarray(W_prjL).T)
    W_fc3T = f(np.asarray(W_fc3).T)
    AGE = f(AGE_inx)
    Endx = np.ascontiguousarray(np.asarray(Endx, np.int32))
    outW = np.asarray(outW, np.float32)
    common = dict(
        AGE=AGE, W_ageT=W_ageT, b_age=f(b_age), W_prjTT=W_prjTT,
        b_prjT=f(b_prjT), W_prjLT=W_prjLT, b_prjL=f(b_prjL), outb=f(outb),
        W_fc3T=W_fc3T, b_fc3=f(b_fc3),
    )
    in_maps = []
    for c in range(NCORES):
        in_maps.append(dict(
            common,
            tT=np.ascontiguousarray(tT[:, NI * c:NI * (c + 1)]),
            endx=np.ascontiguousarray(Endx[R * c:R * (c + 1)]),
            outWT=f(outW[R * c:R * (c + 1)].T),
        ))
    return in_maps


_CACHED_NC = None


def kernel(**inputs):
    global _CACHED_NC
    if _CACHED_NC is None:
        _CACHED_NC = build_bass()
    in_maps = make_in_maps(**inputs)
    res = bass_utils.run_bass_kernel_spmd(
        _CACHED_NC, in_maps, core_ids=list(range(NCORES)))
    o_c = np.concatenate([r["o_c"] for r in res.results], axis=0)
    o_f = np.concatenate([r["o_f"] for r in res.results], axis=0)
    return (o_c, o_f)


if __name__ == "__main__":
    nc = build_bass()
    print("build OK; instructions:",
          sum(len(bb.instructions) for bb in nc.main_func.blocks))


# revision 7
# speedup vs baseline: 1.9649x; 1.9649x over previous
"""Trainium2 Bass kernel for nn_Net_274877907022.

Math (see reference):
    AGE_E = lintrans(AGE_inx @ W_age.T + b_age)        row-wise minmax + L2
    h     = tanh(t @ W_prjT.T + b_prjT)                 [N, 256]
    E     = tanh(AGE_E[Endx] @ W_prjL.T + b_prjL)       [M, 256]
    o_c   = h @ (E * outW).T + outb                     [N, M]
    o_f   = h @ W_fc3.T + b_fc3                         [N, 50]

Key optimizations:
  - lintrans is purely row-wise, so it is computed only on the 4096 gathered
    rows AGE_inx[Endx] (4.8 GFLOP) instead of all 20000 rows (23.6 GFLOP).
  - All GEMMs run with bf16 operands (fp32 matmul is LOW_HIGH dual-pass on
    trn2, ~4-6x slower); accumulation stays fp32 in PSUM, all normalization /
    bias math stays fp32.
  - outb is added during the PSUM->SBUF copy (DVE fused) or by a GPSIMD pass,
    not on the PE.

Sharding over 8 cores:
  - E-branch sharded by Endx: each core gathers 512 rows, computes its
    G^T = (E*outW).T shard [256, 512] in bf16, AllGather -> [2048, 512].
  - h / o_c / o_f data-parallel over N: each core handles 1024 rows of t.
"""

import numpy as np
import sys
from contextlib import ExitStack

for p in ("/opt/trn_rl_repo",):
    if p not in sys.path:
        sys.path.insert(0, p)

import concourse.bass as bass
import concourse.mybir as mybir
import concourse.tile as tile
from concourse import bass_utils
from concourse.masks import make_identity

F32 = mybir.dt.float32
BF16 = mybir.dt.bfloat16
I32 = mybir.dt.int32

NCORES = 8
N, NODES, DIN, HID, M, OUT_FE = 8192, 20000, 768, 256, 4096, 50
NI = N // NCORES          # 1024 t-rows per core
R = M // NCORES           # 512 gathered rows per core
P = 128
DTILES = DIN // P         # 6
JT = HID // P             # 2
RT = R // P               # 4
IT = NI // P              # 8


def build_program(tc: tile.TileContext, io: dict):
    """Emit the kernel IR. `io` maps logical names to DRAM APs."""
    nc = tc.nc

    tT = io["tT"]            # [768, 1024] bf16
    AGE = io["AGE"]          # [20000, 768] bf16
    endx = io["endx"]        # [512] int32
    W_ageT = io["W_ageT"]    # [768, 768] bf16
    b_age = io["b_age"]      # [768] bf16
    W_prjTT = io["W_prjTT"]  # [768, 256] bf16
    b_prjT = io["b_prjT"]    # [256] f32
    W_prjLT = io["W_prjLT"]  # [768, 256] bf16
    b_prjL = io["b_prjL"]    # [256] f32
    outWT = io["outWT"]      # [256, 512] f32   this core's outW shard, transposed
    outb = io["outb"]        # [4096] f32
    W_fc3T = io["W_fc3T"]    # [256, 50] bf16
    b_fc3 = io["b_fc3"]      # [50] bf16
    o_c = io["o_c"]          # [1024, 4096] f32 out
    o_f = io["o_f"]          # [1024, 50] f32 out

    ctx_p = ExitStack()      # whole-kernel pools
    persist = ctx_p.enter_context(tc.tile_pool(name="persist", bufs=1))
    dram = ctx_p.enter_context(tc.tile_pool(name="dram", bufs=1, space="DRAM"))

    ctx_e = ExitStack()      # pools for the E/h/o_f phases
    arows = ctx_e.enter_context(tc.tile_pool(name="arows", bufs=2))
    zpool = ctx_e.enter_context(tc.tile_pool(name="zpool", bufs=2))
    sqpool = ctx_e.enter_context(tc.tile_pool(name="sqpool", bufs=2))
    stats = ctx_e.enter_context(tc.tile_pool(name="stats", bufs=8))
    psum_t = ctx_e.enter_context(tc.tile_pool(name="psum_t", bufs=2, space="PSUM"))
    psum_mm = ctx_e.enter_context(tc.tile_pool(name="psum_mm", bufs=2, space="PSUM"))

    # ---------------- constants / weights into SBUF ----------------
    ident_bf = persist.tile([P, P], BF16, tag="ident_bf")
    make_identity(nc, ident_bf)
    ident_f32 = persist.tile([P, P], F32, tag="ident_f32")
    make_identity(nc, ident_f32)
    ones_bf = persist.tile([1, P], BF16, tag="ones_bf")
    nc.vector.memset(ones_bf[:], 1.0)

    b_age_row = persist.tile([1, DIN], BF16, tag="b_age_row")
    nc.sync.dma_start(b_age_row[:], b_age[None, :])
    bfc3_row = persist.tile([1, OUT_FE], BF16, tag="bfc3_row")
    nc.sync.dma_start(bfc3_row[:], b_fc3[None, :])
    outb_bc = persist.tile([P, M], F32, tag="outb_bc")
    nc.sync.dma_start(outb_bc[:], outb[None, :].to_broadcast((P, M)))

    b_prjT_p = []
    b_prjL_p = []
    for j in range(JT):
        bt = persist.tile([P, 1], F32, tag=f"b_prjT_{j}", name=f"b_prjT_{j}")
        nc.sync.dma_start(bt[:], b_prjT[j * P:(j + 1) * P][:, None])
        b_prjT_p.append(bt)
        bl = persist.tile([P, 1], F32, tag=f"b_prjL_{j}", name=f"b_prjL_{j}")
        nc.sync.dma_start(bl[:], b_prjL[j * P:(j + 1) * P][:, None])
        b_prjL_p.append(bl)

    idx_tiles = []
    for rt in range(RT):
        it_ = persist.tile([P, 1], I32, tag=f"idx_{rt}", name=f"idx_{rt}")
        nc.sync.dma_start(it_[:], endx[rt * P:(rt + 1) * P][:, None])
        idx_tiles.append(it_)

    W_ageT_sb = []
    for d in range(DTILES):
        w = persist.tile([P, DIN], BF16, tag=f"W_ageT_{d}", name=f"W_ageT_{d}")
        nc.sync.dma_start(w[:], W_ageT[d * P:(d + 1) * P, :])
        W_ageT_sb.append(w)
    W_prjLT_sb = []
    W_prjTT_sb = []
    for d in range(DTILES):
        w = persist.tile([P, HID], BF16, tag=f"W_prjLT_{d}", name=f"W_prjLT_{d}")
        nc.sync.dma_start(w[:], W_prjLT[d * P:(d + 1) * P, :])
        W_prjLT_sb.append(w)
        w2 = persist.tile([P, HID], BF16, tag=f"W_prjTT_{d}", name=f"W_prjTT_{d}")
        nc.sync.dma_start(w2[:], W_prjTT[d * P:(d + 1) * P, :])
        W_prjTT_sb.append(w2)
    outWT_sb = []
    W_fc3T_sb = []
    for j in range(JT):
        w = persist.tile([P, R], F32, tag=f"outWT_{j}", name=f"outWT_{j}")
        nc.sync.dma_start(w[:], outWT[j * P:(j + 1) * P, :])
        outWT_sb.append(w)
        w2 = persist.tile([P, OUT_FE], BF16, tag=f"W_fc3T_{j}", name=f"W_fc3T_{j}")
        nc.sync.dma_start(w2[:], W_fc3T[j * P:(j + 1) * P, :])
        W_fc3T_sb.append(w2)

    tT_sb = []
    for d in range(DTILES):
        w = persist.tile([P, NI], BF16, tag=f"tT_{d}", name=f"tT_{d}")
        nc.sync.dma_start(w[:], tT[d * P:(d + 1) * P, :])
        tT_sb.append(w)

    # ---------------- E branch ----------------
    # gather rows (bf16) and transpose them: arT[d] = AGE_rows^T [128(d), 512(r)]
    arT = []
    for d in range(DTILES):
        a = persist.tile([P, R], BF16, tag=f"arT_{d}", name=f"arT_{d}")
        arT.append(a)
    for rt in range(RT):
        ar = arows.tile([P, DIN], BF16, tag="ar", name=f"ar_{rt}")
        nc.gpsimd.indirect_dma_start(
            out=ar[:],
            out_offset=None,
            in_=AGE[:],
            in_offset=bass.IndirectOffsetOnAxis(ap=idx_tiles[rt][:, :1], axis=0),
        )
        for d in range(DTILES):
            pt = psum_t.tile([P, P], BF16, tag="ptb", name=f"ptA_{rt}_{d}")
            nc.tensor.transpose(pt[:], ar[:, d * P:(d + 1) * P], ident_bf[:])
            nc.any.tensor_copy(arT[d][:, rt * P:(rt + 1) * P], pt[:])

    # z = AGE_rows @ W_age^T + b_age (natural layout, fp32 accum), lintrans,
    # then transpose z_norm into znT[d] (bf16) [128(d), 512(r)]
    znT = []
    for d in range(DTILES):
        z = persist.tile([P, R], BF16, tag=f"znT_{d}", name=f"znT_{d}")
        znT.append(z)

    for rt in range(RT):
        z_sb = zpool.tile([P, DIN], F32, tag="z_sb", name=f"z_{rt}")
        for ch in range(2):  # two 384-wide output chunks
            zp = psum_mm.tile([P, 512], F32, tag="mm", name=f"zp_{rt}_{ch}")
            zps = zp[:, :384]
            nc.tensor.matmul(zps, ones_bf[:], b_age_row[:, ch * 384:(ch + 1) * 384],
                             start=True, stop=False)
            for d in range(DTILES):
                nc.tensor.matmul(
                    zps,
                    arT[d][:, rt * P:(rt + 1) * P],
                    W_ageT_sb[d][:, ch * 384:(ch + 1) * 384],
                    start=False, stop=(d == DTILES - 1),
                )
            nc.any.tensor_copy(z_sb[:, ch * 384:(ch + 1) * 384], zps)

        zmin = stats.tile([P, 1], F32, tag="zmin", name=f"zmin_{rt}")
        zmax = stats.tile([P, 1], F32, tag="zmax", name=f"zmax_{rt}")
        rng = stats.tile([P, 1], F32, tag="rng", name=f"rng_{rt}")
        rsc = stats.tile([P, 1], F32, tag="rsc", name=f"rsc_{rt}")
        ssq = stats.tile([P, 1], F32, tag="ssq", name=f"ssq_{rt}")
        nrm = stats.tile([P, 1], F32, tag="nrm", name=f"nrm_{rt}")
        rnm = stats.tile([P, 1], F32, tag="rnm", name=f"rnm_{rt}")
        nc.vector.tensor_reduce(zmin[:], z_sb[:], axis=mybir.AxisListType.X,
                                op=mybir.AluOpType.min)
        nc.vector.tensor_reduce(zmax[:], z_sb[:], axis=mybir.AxisListType.X,
                                op=mybir.AluOpType.max)
        nc.vector.tensor_tensor(rng[:], zmax[:], zmin[:],
                                op=mybir.AluOpType.subtract)
        nc.vector.reciprocal(rsc[:], rng[:])
        # z01 = (z - zmin) * rsc     (in place, fp32)
        nc.vector.tensor_scalar(z_sb[:], z_sb[:], zmin[:], rsc[:],
                                mybir.AluOpType.subtract, mybir.AluOpType.mult)
        sq = sqpool.tile([P, DIN], F32, tag="sq", name=f"sq_{rt}")
        nc.scalar.activation(sq[:], z_sb[:], mybir.ActivationFunctionType.Square,
                             accum_out=ssq[:])
        nc.scalar.sqrt(nrm[:], ssq[:])
        nc.vector.tensor_scalar_max(nrm[:], nrm[:], 1e-12)
        nc.vector.reciprocal(rnm[:], nrm[:])
        nc.vector.tensor_scalar_mul(z_sb[:], z_sb[:], rnm[:])
        # transpose z_norm (fp32 PE transpose), cast to bf16 on the copy out
        for d in range(DTILES):
            pt = psum_t.tile([P, P], F32, tag="ptf", name=f"ptZ_{rt}_{d}")
            nc.tensor.transpose(pt[:], z_sb[:, d * P:(d + 1) * P], ident_f32[:])
            nc.any.tensor_copy(znT[d][:, rt * P:(rt + 1) * P], pt[:])

    # E^T = tanh(W_prjL @ z_norm^T + b_prjL); G^T = E^T * outW^T  (bf16 out)
    ag_in = dram.tile([HID, R], BF16, name="ag_in")
    for j in range(JT):
        ep = psum_mm.tile([P, 512], F32, tag="mm", name=f"ep_{j}")
        for d in range(DTILES):
            nc.tensor.matmul(ep[:], W_prjLT_sb[d][:, j * P:(j + 1) * P], znT[d][:],
                             start=(d == 0), stop=(d == DTILES - 1))
        eT = persist.tile([P, R], F32, tag=f"eT_{j}", name=f"eT_{j}")
        nc.scalar.activation(eT[:], ep[:], mybir.ActivationFunctionType.Tanh,
                             bias=b_prjL_p[j][:, :1], scale=1.0)
        gT = persist.tile([P, R], BF16, tag=f"gT_{j}", name=f"gT_{j}")
        nc.vector.tensor_tensor(gT[:], eT[:], outWT_sb[j][:],
                                op=mybir.AluOpType.mult)
        nc.sync.dma_start(ag_in[j * P:(j + 1) * P, :], gT[:])

    # AllGather the G^T shards (bf16): [256,512] per core -> [2048,512]
    ag_out = dram.tile([NCORES * HID, R], BF16, name="ag_out", addr_space="Shared")
    nc.gpsimd.collective_compute(
        "AllGather",
        mybir.AluOpType.bypass,
        replica_groups=[list(range(NCORES))],
        ins=[ag_in.opt()],
        outs=[ag_out.opt()],
    )

    # ---------------- h branch (independent of E branch) ----------------
    hT = []
    for j in range(JT):
        h = persist.tile([P, NI], BF16, tag=f"hT_{j}", name=f"hT_{j}")
        hT.append(h)
    for j in range(JT):
        for ih in range(NI // 512):
            hp = psum_mm.tile([P, 512], F32, tag="mm", name=f"hp_{j}_{ih}")
            for d in range(DTILES):
                nc.tensor.matmul(hp[:], W_prjTT_sb[d][:, j * P:(j + 1) * P],
                                 tT_sb[d][:, ih * 512:(ih + 1) * 512],
                                 start=(d == 0), stop=(d == DTILES - 1))
            nc.scalar.activation(hT[j][:, ih * 512:(ih + 1) * 512], hp[:],
                                 mybir.ActivationFunctionType.Tanh,
                                 bias=b_prjT_p[j][:, :1], scale=1.0)

    # ---------------- o_f = h @ W_fc3^T + b_fc3 ----------------
    of_all = persist.tile([P, IT, OUT_FE], F32, tag="of_all")
    for it in range(IT):
        fp = psum_mm.tile([P, 512], F32, tag="mm", name=f"fp_{it}")
        fps = fp[:, :OUT_FE]
        nc.tensor.matmul(fps, ones_bf[:], bfc3_row[:], start=True, stop=False)
        for j in range(JT):
            nc.tensor.matmul(fps, hT[j][:, it * P:(it + 1) * P], W_fc3T_sb[j][:],
                             start=False, stop=(j == JT - 1))
        nc.any.tensor_copy(of_all[:, it, :], fps)
    nc.sync.dma_start(o_f.rearrange("(t p) f -> p t f", p=P), of_all[:])

    # read back the gathered G^T chunks (bf16)
    GT = [[None] * JT for _ in range(NCORES)]
    for c in range(NCORES):
        for j in range(JT):
            g = persist.tile([P, R], BF16, tag=f"GT_{c}_{j}", name=f"GT_{c}_{j}")
            nc.sync.dma_start(g[:], ag_out[(c * JT + j) * P:(c * JT + j + 1) * P, :])
            GT[c][j] = g

    ctx_e.close()

    # ---------------- o_c = h @ G^T + outb ----------------
    ctx_o = ExitStack()
    ocstage = ctx_o.enter_context(tc.tile_pool(name="ocstage", bufs=2))
    psum_oc = ctx_o.enter_context(tc.tile_pool(name="psum_oc", bufs=4, space="PSUM"))

    for it in range(IT):
        for half in range(2):
            stage = ocstage.tile([P, 2048], F32, tag="ocs", name=f"ocs_{it}_{half}")
            ops = [psum_oc.tile([P, 512], F32, tag="oc", name=f"op_{it}_{half}_{mc}")
                   for mc in range(4)]
            # keep each lhsT loaded across the 4 column chunks
            for j in range(JT):
                for mc in range(4):
                    c = half * 4 + mc
                    nc.tensor.matmul(ops[mc][:], hT[j][:, it * P:(it + 1) * P],
                                     GT[c][j][:], start=(j == 0), stop=(j == JT - 1))
            off = half * 2048
            # outb add fused into the PSUM->SBUF copy on DVE for chunks 0,1;
            # plain ACT copies for chunks 2,3 with a GPSIMD bias pass.
            nc.vector.tensor_tensor(stage[:, 0:512], ops[0][:],
                                    outb_bc[:, off:off + 512],
                                    op=mybir.AluOpType.add)
            nc.vector.tensor_tensor(stage[:, 512:1024], ops[1][:],
                                    outb_bc[:, off + 512:off + 1024],
                                    op=mybir.AluOpType.add)
            nc.scalar.copy(stage[:, 1024:1536], ops[2][:])
            nc.scalar.copy(stage[:, 1536:2048], ops[3][:])
            nc.gpsimd.tensor_tensor(stage[:, 1024:2048], stage[:, 1024:2048],
                                    outb_bc[:, off + 1024:off + 2048],
                                    op=mybir.AluOpType.add)
            nc.sync.dma_start(
                o_c[it * P:(it + 1) * P, half * 2048:(half + 1) * 2048], stage[:])

    ctx_o.close()
    ctx_p.close()


def legalize_sync(nc: bass.Bass):
    """The container's walrus accepts at most one sync wait and one sync
    update per instruction. Split extras onto adjacent NoOps (same engine,
    in-order queue => identical semantics)."""
    cnt = 0
    for func in nc.m.functions:
        for bb in func.blocks:
            new_insts = []
            for inst in bb.instructions:
                si = getattr(inst, "sync_info", None)
                waits = list(si.on_wait) if si is not None and si.on_wait else []
                if si is not None and len(waits) > 1:
                    for w in waits[:-1]:
                        cnt += 1
                        new_insts.append(mybir.InstNoOp(
                            name=f"{inst.name}_sw{cnt}",
                            sync_info=mybir.SyncInfo(on_wait=[w], on_update=[]),
                            bass_nofuse=True,
                            engine=inst.engine,
                        ))
                    si.on_wait = [waits[-1]]
                new_insts.append(inst)
                upds = list(si.on_update) if si is not None and si.on_update else []
                if si is not None and len(upds) > 1:
                    si.on_update = [upds[0]]
                    for u in upds[1:]:
                        cnt += 1
                        new_insts.append(mybir.InstNoOp(
                            name=f"{inst.name}_su{cnt}",
                            sync_info=mybir.SyncInfo(on_wait=[], on_update=[u]),
                            bass_nofuse=True,
                            engine=inst.engine,
                        ))
            bb.instructions[:] = new_insts
    return cnt


def build_bass() -> bass.Bass:
    nc = bass.Bass(trn_type="TRN2", num_devices=NCORES)
    io = {}
    io["tT"] = nc.dram_tensor("tT", [DIN, NI], BF16, kind="ExternalInput").ap()
    io["AGE"] = nc.dram_tensor("AGE", [NODES, DIN], BF16, kind="ExternalInput").ap()
    io["endx"] = nc.dram_tensor("endx", [R], I32, kind="ExternalInput").ap()
    io["W_ageT"] = nc.dram_tensor("W_ageT", [DIN, DIN], BF16, kind="ExternalInput").ap()
    io["b_age"] = nc.dram_tensor("b_age", [DIN], BF16, kind="ExternalInput").ap()
    io["W_prjTT"] = nc.dram_tensor("W_prjTT", [DIN, HID], BF16, kind="ExternalInput").ap()
    io["b_prjT"] = nc.dram_tensor("b_prjT", [HID], F32, kind="ExternalInput").ap()
    io["W_prjLT"] = nc.dram_tensor("W_prjLT", [DIN, HID], BF16, kind="ExternalInput").ap()
    io["b_prjL"] = nc.dram_tensor("b_prjL", [HID], F32, kind="ExternalInput").ap()
    io["outWT"] = nc.dram_tensor("outWT", [HID, R], F32, kind="ExternalInput").ap()
    io["outb"] = nc.dram_tensor("outb", [M], F32, kind="ExternalInput").ap()
    io["W_fc3T"] = nc.dram_tensor("W_fc3T", [HID, OUT_FE], BF16, kind="ExternalInput").ap()
    io["b_fc3"] = nc.dram_tensor("b_fc3", [OUT_FE], BF16, kind="ExternalInput").ap()
    io["o_c"] = nc.dram_tensor("o_c", [NI, M], F32, kind="ExternalOutput").ap()
    io["o_f"] = nc.dram_tensor("o_f", [NI, OUT_FE], F32, kind="ExternalOutput").ap()
    with tile.TileContext(nc) as tc:
        build_program(tc, io)
    legalize_sync(nc)
    return nc


def make_in_maps(t, AGE_inx, Endx, W_age, b_age, W_prjT, b_prjT, W_prjL,
                 b_prjL, outW, outb, W_fc3, b_fc3):
    import ml_dtypes
    bf = lambda a: np.ascontiguousarray(np.asarray(a, np.float32).astype(ml_dtypes.bfloat16))
    f = lambda a: np.ascontiguousarray(np.asarray(a, np.float32))
    tT = bf(np.asarray(t, np.float32).T)          # [768, 8192] bf16
    Endx = np.ascontiguousarray(np.asarray(Endx, np.int32))
    outW = np.asarray(outW, np.float32)
    common = dict(
        AGE=bf(AGE_inx),
        W_ageT=bf(np.asarray(W_age, np.float32).T),
        b_age=bf(b_age),
        W_prjTT=bf(np.asarray(W_prjT, np.float32).T),
        b_prjT=f(b_prjT),
        W_prjLT=bf(np.asarray(W_prjL, np.float32).T),
        b_prjL=f(b_prjL),
        outb=f(outb),
        W_fc3T=bf(np.asarray(W_fc3, np.float32).T),
        b_fc3=bf(b_fc3),
    )
    in_maps = []
    for c in range(NCORES):
        in_maps.append(dict(
            common,
            tT=np.ascontiguousarray(tT[:, NI * c:NI * (c + 1)]),
            endx=np.ascontiguousarray(Endx[R * c:R * (c + 1)]),
            outWT=f(outW[R * c:R * (c + 1)].T),
        ))
    return in_maps


_CACHED_NC = None


def kernel(**inputs):
    global _CACHED_NC
    if _CACHED_NC is None:
        _CACHED_NC = build_bass()
    in_maps = make_in_maps(**inputs)
    res = bass_utils.run_bass_kernel_spmd(
        _CACHED_NC, in_maps, core_ids=list(range(NCORES)))
    o_c = np.concatenate([r["o_c"] for r in res.results], axis=0)
    o_f = np.concatenate([r["o_f"] for r in res.results], axis=0)
    return (o_c, o_f)


if __name__ == "__main__":
    nc = build_bass()
    print("build OK; instructions:",
          sum(len(bb.instructions) for bb in nc.main_func.blocks))


# revision 14
# speedup vs baseline: 2.0536x; 1.0452x over previous
"""Trainium2 Bass kernel for nn_Net_274877907022.

Math (see reference):
    AGE_E = lintrans(AGE_inx @ W_age.T + b_age)        row-wise minmax + L2
    h     = tanh(t @ W_prjT.T + b_prjT)                 [N, 256]
    E     = tanh(AGE_E[Endx] @ W_prjL.T + b_prjL)       [M, 256]
    o_c   = h @ (E * outW).T + outb                     [N, M]
    o_f   = h @ W_fc3.T + b_fc3                         [N, 50]

Key optimizations:
  - lintrans is purely row-wise, so it is computed only on the 4096 gathered
    rows AGE_inx[Endx] (4.8 GFLOP) instead of all 20000 rows (23.6 GFLOP).
  - All GEMMs run with bf16 operands (fp32 matmul is LOW_HIGH dual-pass on
    trn2, ~4-6x slower); accumulation stays fp32 in PSUM, all normalization /
    bias math stays fp32.
  - outb is added during the PSUM->SBUF copy (DVE fused) or by a GPSIMD pass,
    not on the PE.

Sharding over 8 cores:
  - E-branch sharded by Endx: each core gathers 512 rows, computes its
    G^T = (E*outW).T shard [256, 512] in bf16, AllGather -> [2048, 512].
  - h / o_c / o_f data-parallel over N: each core handles 1024 rows of t.
"""

import numpy as np
import sys
from contextlib import ExitStack

for p in ("/opt/trn_rl_repo",):
    if p not in sys.path:
        sys.path.insert(0, p)

import concourse.bass as bass
import concourse.mybir as mybir
import concourse.tile as tile
from concourse import bass_utils
from concourse.masks import make_identity

F32 = mybir.dt.float32
BF16 = mybir.dt.bfloat16
I32 = mybir.dt.int32

NCORES = 8
N, NODES, DIN, HID, M, OUT_FE = 8192, 20000, 768, 256, 4096, 50
NI = N // NCORES          # 1024 t-rows per core
R = M // NCORES           # 512 gathered rows per core
P = 128
DTILES = DIN // P         # 6
JT = HID // P             # 2
RT = R // P               # 4
IT = NI // P              # 8


def build_program(tc: tile.TileContext, io: dict):
    """Emit the kernel IR. `io` maps logical names to DRAM APs."""
    nc = tc.nc

    tT = io["tT"]            # [768, 1024] bf16
    AGE = io["AGE"]          # [20000, 768] bf16
    endx = io["endx"]        # [512] int32
    W_ageT = io["W_ageT"]    # [768, 768] bf16
    b_age = io["b_age"]      # [768] bf16
    W_prjTT = io["W_prjTT"]  # [768, 256] bf16
    b_prjT = io["b_prjT"]    # [256] f32
    W_prjLT = io["W_prjLT"]  # [768, 256] bf16
    b_prjL = io["b_prjL"]    # [256] f32
    outWT = io["outWT"]      # [256, 512] f32   this core's outW shard, transposed
    outb = io["outb"]        # [4096] f32
    outb_h = io["outb_h"]    # [4096] f16
    W_fc3T = io["W_fc3T"]    # [256, 50] bf16
    b_fc3 = io["b_fc3"]      # [50] bf16
    o_c = io["o_c"]          # [1024, 4096] f32 out
    o_f = io["o_f"]          # [1024, 50] f32 out

    ctx_p = ExitStack()      # whole-kernel pools
    persist = ctx_p.enter_context(tc.tile_pool(name="persist", bufs=1))
    dram = ctx_p.enter_context(tc.tile_pool(name="dram", bufs=1, space="DRAM"))

    ctx_e = ExitStack()      # pools for the E/h/o_f phases
    arows = ctx_e.enter_context(tc.tile_pool(name="arows", bufs=2))
    zpool = ctx_e.enter_context(tc.tile_pool(name="zpool", bufs=2))
    sqpool = ctx_e.enter_context(tc.tile_pool(name="sqpool", bufs=2))
    stats = ctx_e.enter_context(tc.tile_pool(name="stats", bufs=8))
    psum_t = ctx_e.enter_context(tc.tile_pool(name="psum_t", bufs=2, space="PSUM"))
    psum_mm = ctx_e.enter_context(tc.tile_pool(name="psum_mm", bufs=2, space="PSUM"))

    import os
    if os.environ.get("K_WARM_AG", "1") == "1":
        # Tiny AllGather first: acts as a rank barrier + collective warm-up, so
        # the real AllGather later doesn't pay rank-skew / first-call setup.
        warm_in = dram.tile([1, 256], BF16, name="warm_in")
        warm_out = dram.tile([NCORES, 256], BF16, name="warm_out",
                             addr_space="Shared")
        warm_sb = persist.tile([1, 256], BF16, tag="warm_sb")
        nc.vector.memset(warm_sb[:], 0.0)
        nc.sync.dma_start(warm_in[:], warm_sb[:])
        nc.gpsimd.collective_compute(
            "AllGather",
            mybir.AluOpType.bypass,
            replica_groups=[list(range(NCORES))],
            ins=[warm_in.opt()],
            outs=[warm_out.opt()],
        )

    # ---------------- constants / weights into SBUF ----------------
    ident_bf = persist.tile([P, P], BF16, tag="ident_bf")
    make_identity(nc, ident_bf)
    ident_f32 = persist.tile([P, P], F32, tag="ident_f32")
    make_identity(nc, ident_f32)
    ones_bf = persist.tile([1, P], BF16, tag="ones_bf")
    nc.vector.memset(ones_bf[:], 1.0)
    ones_h16 = persist.tile([1, P], mybir.dt.float16, tag="ones_h16")
    nc.vector.memset(ones_h16[:], 1.0)
    outb_h_row = persist.tile([1, M], mybir.dt.float16, tag="outb_h_row")
    nc.sync.dma_start(outb_h_row[:], outb_h[None, :])

    b_age_row = persist.tile([1, DIN], BF16, tag="b_age_row")
    nc.sync.dma_start(b_age_row[:], b_age[None, :])
    bfc3_row = persist.tile([1, OUT_FE], BF16, tag="bfc3_row")
    nc.sync.dma_start(bfc3_row[:], b_fc3[None, :])
    outb_bc = persist.tile([P, M], F32, tag="outb_bc")
    nc.sync.dma_start(outb_bc[:], outb[None, :].to_broadcast((P, M)))

    b_prjT_p = []
    b_prjL_p = []
    for j in range(JT):
        bt = persist.tile([P, 1], F32, tag=f"b_prjT_{j}", name=f"b_prjT_{j}")
        nc.sync.dma_start(bt[:], b_prjT[j * P:(j + 1) * P][:, None])
        b_prjT_p.append(bt)
        bl = persist.tile([P, 1], F32, tag=f"b_prjL_{j}", name=f"b_prjL_{j}")
        nc.sync.dma_start(bl[:], b_prjL[j * P:(j + 1) * P][:, None])
        b_prjL_p.append(bl)

    idx_tiles = []
    for rt in range(RT):
        it_ = persist.tile([P, 1], I32, tag=f"idx_{rt}", name=f"idx_{rt}")
        nc.sync.dma_start(it_[:], endx[rt * P:(rt + 1) * P][:, None])
        idx_tiles.append(it_)

    W_ageT_sb = []
    for d in range(DTILES):
        w = persist.tile([P, DIN], BF16, tag=f"W_ageT_{d}", name=f"W_ageT_{d}")
        nc.sync.dma_start(w[:], W_ageT[d * P:(d + 1) * P, :])
        W_ageT_sb.append(w)
    W_prjLT_sb = []
    W_prjTT_sb = []
    for d in range(DTILES):
        w = persist.tile([P, HID], BF16, tag=f"W_prjLT_{d}", name=f"W_prjLT_{d}")
        nc.sync.dma_start(w[:], W_prjLT[d * P:(d + 1) * P, :])
        W_prjLT_sb.append(w)
        w2 = persist.tile([P, HID], BF16, tag=f"W_prjTT_{d}", name=f"W_prjTT_{d}")
        nc.sync.dma_start(w2[:], W_prjTT[d * P:(d + 1) * P, :])
        W_prjTT_sb.append(w2)
    outWT_sb = []
    W_fc3T_sb = []
    for j in range(JT):
        w = persist.tile([P, R], F32, tag=f"outWT_{j}", name=f"outWT_{j}")
        nc.sync.dma_start(w[:], outWT[j * P:(j + 1) * P, :])
        outWT_sb.append(w)
        w2 = persist.tile([P, OUT_FE], BF16, tag=f"W_fc3T_{j}", name=f"W_fc3T_{j}")
        nc.sync.dma_start(w2[:], W_fc3T[j * P:(j + 1) * P, :])
        W_fc3T_sb.append(w2)

    tT_sb = []
    for d in range(DTILES):
        w = persist.tile([P, NI], BF16, tag=f"tT_{d}", name=f"tT_{d}")
        nc.sync.dma_start(w[:], tT[d * P:(d + 1) * P, :])
        tT_sb.append(w)

    # ---------------- E branch ----------------
    # gather rows (bf16) and transpose them: arT[d] = AGE_rows^T [128(d), 512(r)]
    arT = []
    for d in range(DTILES):
        a = persist.tile([P, R], BF16, tag=f"arT_{d}", name=f"arT_{d}")
        arT.append(a)
    for rt in range(RT):
        ar = arows.tile([P, DIN], BF16, tag="ar", name=f"ar_{rt}")
        nc.gpsimd.indirect_dma_start(
            out=ar[:],
            out_offset=None,
            in_=AGE[:],
            in_offset=bass.IndirectOffsetOnAxis(ap=idx_tiles[rt][:, :1], axis=0),
        )
        for d in range(DTILES):
            pt = psum_t.tile([P, P], BF16, tag="ptb", name=f"ptA_{rt}_{d}")
            nc.tensor.transpose(pt[:], ar[:, d * P:(d + 1) * P], ident_bf[:])
            nc.any.tensor_copy(arT[d][:, rt * P:(rt + 1) * P], pt[:])

    # z = AGE_rows @ W_age^T + b_age (natural layout, fp32 accum), lintrans,
    # then transpose z_norm into znT[d] (bf16) [128(d), 512(r)]
    znT = []
    for d in range(DTILES):
        z = persist.tile([P, R], BF16, tag=f"znT_{d}", name=f"znT_{d}")
        znT.append(z)

    for rt in range(RT):
        z_sb = zpool.tile([P, DIN], F32, tag="z_sb", name=f"z_{rt}")
        for ch in range(2):  # two 384-wide output chunks
            zp = psum_mm.tile([P, 512], F32, tag="mm", name=f"zp_{rt}_{ch}")
            zps = zp[:, :384]
            nc.tensor.matmul(zps, ones_bf[:], b_age_row[:, ch * 384:(ch + 1) * 384],
                             start=True, stop=False)
            for d in range(DTILES):
                nc.tensor.matmul(
                    zps,
                    arT[d][:, rt * P:(rt + 1) * P],
                    W_ageT_sb[d][:, ch * 384:(ch + 1) * 384],
                    start=False, stop=(d == DTILES - 1),
                )
            nc.any.tensor_copy(z_sb[:, ch * 384:(ch + 1) * 384], zps)

        zmin = stats.tile([P, 1], F32, tag="zmin", name=f"zmin_{rt}")
        zmax = stats.tile([P, 1], F32, tag="zmax", name=f"zmax_{rt}")
        rng = stats.tile([P, 1], F32, tag="rng", name=f"rng_{rt}")
        rsc = stats.tile([P, 1], F32, tag="rsc", name=f"rsc_{rt}")
        ssq = stats.tile([P, 1], F32, tag="ssq", name=f"ssq_{rt}")
        nrm = stats.tile([P, 1], F32, tag="nrm", name=f"nrm_{rt}")
        rnm = stats.tile([P, 1], F32, tag="rnm", name=f"rnm_{rt}")
        nc.vector.tensor_reduce(zmin[:], z_sb[:], axis=mybir.AxisListType.X,
                                op=mybir.AluOpType.min)
        nc.vector.tensor_reduce(zmax[:], z_sb[:], axis=mybir.AxisListType.X,
                                op=mybir.AluOpType.max)
        nc.vector.tensor_tensor(rng[:], zmax[:], zmin[:],
                                op=mybir.AluOpType.subtract)
        nc.vector.reciprocal(rsc[:], rng[:])
        # z01 = (z - zmin) * rsc     (in place, fp32)
        nc.vector.tensor_scalar(z_sb[:], z_sb[:], zmin[:], rsc[:],
                                mybir.AluOpType.subtract, mybir.AluOpType.mult)
        sq = sqpool.tile([P, DIN], F32, tag="sq", name=f"sq_{rt}")
        nc.scalar.activation(sq[:], z_sb[:], mybir.ActivationFunctionType.Square,
                             accum_out=ssq[:])
        nc.scalar.sqrt(nrm[:], ssq[:])
        nc.vector.tensor_scalar_max(nrm[:], nrm[:], 1e-12)
        nc.vector.reciprocal(rnm[:], nrm[:])
        nc.vector.tensor_scalar_mul(z_sb[:], z_sb[:], rnm[:])
        # transpose z_norm (fp32 PE transpose), cast to bf16 on the copy out
        for d in range(DTILES):
            pt = psum_t.tile([P, P], F32, tag="ptf", name=f"ptZ_{rt}_{d}")
            nc.tensor.transpose(pt[:], z_sb[:, d * P:(d + 1) * P], ident_f32[:])
            nc.any.tensor_copy(znT[d][:, rt * P:(rt + 1) * P], pt[:])

    # E^T = tanh(W_prjL @ z_norm^T + b_prjL); G^T = E^T * outW^T  (bf16 out)
    ag_in = dram.tile([HID, R], BF16, name="ag_in")
    for j in range(JT):
        ep = psum_mm.tile([P, 512], F32, tag="mm", name=f"ep_{j}")
        for d in range(DTILES):
            nc.tensor.matmul(ep[:], W_prjLT_sb[d][:, j * P:(j + 1) * P], znT[d][:],
                             start=(d == 0), stop=(d == DTILES - 1))
        eT = persist.tile([P, R], F32, tag=f"eT_{j}", name=f"eT_{j}")
        nc.scalar.activation(eT[:], ep[:], mybir.ActivationFunctionType.Tanh,
                             bias=b_prjL_p[j][:, :1], scale=1.0)
        gT = persist.tile([P, R], BF16, tag=f"gT_{j}", name=f"gT_{j}")
        nc.vector.tensor_tensor(gT[:], eT[:], outWT_sb[j][:],
                                op=mybir.AluOpType.mult)
        nc.sync.dma_start(ag_in[j * P:(j + 1) * P, :], gT[:])

    # AllGather the G^T shards (bf16): [256,512] per core -> [2048,512]
    ag_out = dram.tile([NCORES * HID, R], BF16, name="ag_out", addr_space="Shared")
    nc.gpsimd.collective_compute(
        "AllGather",
        mybir.AluOpType.bypass,
        replica_groups=[list(range(NCORES))],
        ins=[ag_in.opt()],
        outs=[ag_out.opt()],
    )

    # ---------------- h branch (independent of E branch) ----------------
    hT = []
    for j in range(JT):
        h = persist.tile([P, NI], BF16, tag=f"hT_{j}", name=f"hT_{j}")
        hT.append(h)
    for j in range(JT):
        for ih in range(NI // 512):
            hp = psum_mm.tile([P, 512], F32, tag="mm", name=f"hp_{j}_{ih}")
            for d in range(DTILES):
                nc.tensor.matmul(hp[:], W_prjTT_sb[d][:, j * P:(j + 1) * P],
                                 tT_sb[d][:, ih * 512:(ih + 1) * 512],
                                 start=(d == 0), stop=(d == DTILES - 1))
            nc.scalar.activation(hT[j][:, ih * 512:(ih + 1) * 512], hp[:],
                                 mybir.ActivationFunctionType.Tanh,
                                 bias=b_prjT_p[j][:, :1], scale=1.0)

    # ---------------- o_f = h @ W_fc3^T + b_fc3 ----------------
    of_all = persist.tile([P, IT, OUT_FE], F32, tag="of_all")
    for it in range(IT):
        fp = psum_mm.tile([P, 512], F32, tag="mm", name=f"fp_{it}")
        fps = fp[:, :OUT_FE]
        nc.tensor.matmul(fps, ones_bf[:], bfc3_row[:], start=True, stop=False)
        for j in range(JT):
            nc.tensor.matmul(fps, hT[j][:, it * P:(it + 1) * P], W_fc3T_sb[j][:],
                             start=False, stop=(j == JT - 1))
        nc.any.tensor_copy(of_all[:, it, :], fps)
    nc.sync.dma_start(o_f.rearrange("(t p) f -> p t f", p=P), of_all[:])

    # read back the gathered G^T chunks (bf16)
    GT = [[None] * JT for _ in range(NCORES)]
    for c in range(NCORES):
        for j in range(JT):
            g = persist.tile([P, R], BF16, tag=f"GT_{c}_{j}", name=f"GT_{c}_{j}")
            nc.sync.dma_start(g[:], ag_out[(c * JT + j) * P:(c * JT + j + 1) * P, :])
            GT[c][j] = g

    ctx_e.close()

    # ---------------- o_c = h @ G^T + outb ----------------
    ctx_o = ExitStack()
    ocstage = ctx_o.enter_context(tc.tile_pool(name="ocstage", bufs=3))
    psum_oc = ctx_o.enter_context(tc.tile_pool(name="psum_oc", bufs=6, space="PSUM"))

    for it in range(IT):
        for half in range(2):
            stage = ocstage.tile([P, 2048], F32, tag="ocs", name=f"ocs_{it}_{half}")
            ops = [psum_oc.tile([P, 512], F32, tag="oc", name=f"op_{it}_{half}_{mc}")
                   for mc in range(4)]
            off = half * 2048
            # chunks 2,3 get outb via a cheap fp16 ones-matmul into PSUM;
            # chunks 0,1 get it exactly (fp32) fused into the DVE copy below.
            for mc in (2, 3):
                nc.tensor.matmul(ops[mc][:], ones_h16[:],
                                 outb_h_row[:, off + mc * 512:off + (mc + 1) * 512],
                                 start=True, stop=False)
            # keep each lhsT loaded across the 4 column chunks
            for j in range(JT):
                for mc in range(4):
                    c = half * 4 + mc
                    nc.tensor.matmul(ops[mc][:], hT[j][:, it * P:(it + 1) * P],
                                     GT[c][j][:],
                                     start=(j == 0 and mc < 2),
                                     stop=(j == JT - 1))
            nc.vector.tensor_tensor(stage[:, 0:512], ops[0][:],
                                    outb_bc[:, off:off + 512],
                                    op=mybir.AluOpType.add)
            nc.vector.tensor_tensor(stage[:, 512:1024], ops[1][:],
                                    outb_bc[:, off + 512:off + 1024],
                                    op=mybir.AluOpType.add)
            nc.scalar.copy(stage[:, 1024:1536], ops[2][:])
            nc.scalar.copy(stage[:, 1536:2048], ops[3][:])
            nc.sync.dma_start(
                o_c[it * P:(it + 1) * P, half * 2048:(half + 1) * 2048], stage[:])

    ctx_o.close()
    ctx_p.close()


def legalize_sync(nc: bass.Bass):
    """The container's walrus accepts at most one sync wait and one sync
    update per instruction. Split extras onto adjacent NoOps (same engine,
    in-order queue => identical semantics)."""
    cnt = 0
    for func in nc.m.functions:
        for bb in func.blocks:
            new_insts = []
            for inst in bb.instructions:
                si = getattr(inst, "sync_info", None)
                waits = list(si.on_wait) if si is not None and si.on_wait else []
                if si is not None and len(waits) > 1:
                    for w in waits[:-1]:
                        cnt += 1
                        new_insts.append(mybir.InstNoOp(
                            name=f"{inst.name}_sw{cnt}",
                            sync_info=mybir.SyncInfo(on_wait=[w], on_update=[]),
                            bass_nofuse=True,
                            engine=inst.engine,
                        ))
                    si.on_wait = [waits[-1]]
                new_insts.append(inst)
                upds = list(si.on_update) if si is not None and si.on_update else []
                if si is not None and len(upds) > 1:
                    si.on_update = [upds[0]]
                    for u in upds[1:]:
                        cnt += 1
                        new_insts.append(mybir.InstNoOp(
                            name=f"{inst.name}_su{cnt}",
                            sync_info=mybir.SyncInfo(on_wait=[], on_update=[u]),
                            bass_nofuse=True,
                            engine=inst.engine,
                        ))
            bb.instructions[:] = new_insts
    return cnt


def build_bass() -> bass.Bass:
    nc = bass.Bass(trn_type="TRN2", num_devices=NCORES)
    io = {}
    io["tT"] = nc.dram_tensor("tT", [DIN, NI], BF16, kind="ExternalInput").ap()
    io["AGE"] = nc.dram_tensor("AGE", [NODES, DIN], BF16, kind="ExternalInput").ap()
    io["endx"] = nc.dram_tensor("endx", [R], I32, kind="ExternalInput").ap()
    io["W_ageT"] = nc.dram_tensor("W_ageT", [DIN, DIN], BF16, kind="ExternalInput").ap()
    io["b_age"] = nc.dram_tensor("b_age", [DIN], BF16, kind="ExternalInput").ap()
    io["W_prjTT"] = nc.dram_tensor("W_prjTT", [DIN, HID], BF16, kind="ExternalInput").ap()
    io["b_prjT"] = nc.dram_tensor("b_prjT", [HID], F32, kind="ExternalInput").ap()
    io["W_prjLT"] = nc.dram_tensor("W_prjLT", [DIN, HID], BF16, kind="ExternalInput").ap()
    io["b_prjL"] = nc.dram_tensor("b_prjL", [HID], F32, kind="ExternalInput").ap()
    io["outWT"] = nc.dram_tensor("outWT", [HID, R], F32, kind="ExternalInput").ap()
    io["outb"] = nc.dram_tensor("outb", [M], F32, kind="ExternalInput").ap()
    io["outb_h"] = nc.dram_tensor("outb_h", [M], mybir.dt.float16,
                                  kind="ExternalInput").ap()
    io["W_fc3T"] = nc.dram_tensor("W_fc3T", [HID, OUT_FE], BF16, kind="ExternalInput").ap()
    io["b_fc3"] = nc.dram_tensor("b_fc3", [OUT_FE], BF16, kind="ExternalInput").ap()
    io["o_c"] = nc.dram_tensor("o_c", [NI, M], F32, kind="ExternalOutput").ap()
    io["o_f"] = nc.dram_tensor("o_f", [NI, OUT_FE], F32, kind="ExternalOutput").ap()
    with tile.TileContext(nc) as tc:
        build_program(tc, io)
    legalize_sync(nc)
    return nc


def make_in_maps(t, AGE_inx, Endx, W_age, b_age, W_prjT, b_prjT, W_prjL,
                 b_prjL, outW, outb, W_fc3, b_fc3):
    import ml_dtypes
    bf = lambda a: np.ascontiguousarray(np.asarray(a, np.float32).astype(ml_dtypes.bfloat16))
    f = lambda a: np.ascontiguousarray(np.asarray(a, np.float32))
    tT = bf(np.asarray(t, np.float32).T)          # [768, 8192] bf16
    Endx = np.ascontiguousarray(np.asarray(Endx, np.int32))
    outW = np.asarray(outW, np.float32)
    common = dict(
        AGE=bf(AGE_inx),
        W_ageT=bf(np.asarray(W_age, np.float32).T),
        b_age=bf(b_age),
        W_prjTT=bf(np.asarray(W_prjT, np.float32).T),
        b_prjT=f(b_prjT),
        W_prjLT=bf(np.asarray(W_prjL, np.float32).T),
        b_prjL=f(b_prjL),
        outb=f(outb),
        outb_h=np.ascontiguousarray(np.asarray(outb, np.float32).astype(np.float16)),
        W_fc3T=bf(np.asarray(W_fc3, np.float32).T),
        b_fc3=bf(b_fc3),
    )
    in_maps = []
    for c in range(NCORES):
        in_maps.append(dict(
            common,
            tT=np.ascontiguousarray(tT[:, NI * c:NI * (c + 1)]),
            endx=np.ascontiguousarray(Endx[R * c:R * (c + 1)]),
            outWT=f(outW[R * c:R * (c + 1)].T),
        ))
    return in_maps


_CACHED_NC = None


def kernel(**inputs):
    global _CACHED_NC
    if _CACHED_NC is None:
        _CACHED_NC = build_bass()
    in_maps = make_in_maps(**inputs)
    res = bass_utils.run_bass_kernel_spmd(
        _CACHED_NC, in_maps, core_ids=list(range(NCORES)))
    o_c = np.concatenate([r["o_c"] for r in res.results], axis=0)
    o_f = np.concatenate([r["o_f"] for r in res.results], axis=0)
    return (o_c, o_f)


if __name__ == "__main__":
    nc = build_bass()
    print("build OK; instructions:",
          sum(len(bb.instructions) for bb in nc.main_func.blocks))


# revision 17
# speedup vs baseline: 2.2758x; 1.1082x over previous
"""Trainium2 Bass kernel for nn_Net_274877907022.

Math (see reference):
    AGE_E = lintrans(AGE_inx @ W_age.T + b_age)        row-wise minmax + L2
    h     = tanh(t @ W_prjT.T + b_prjT)                 [N, 256]
    E     = tanh(AGE_E[Endx] @ W_prjL.T + b_prjL)       [M, 256]
    o_c   = h @ (E * outW).T + outb                     [N, M]
    o_f   = h @ W_fc3.T + b_fc3                         [N, 50]

Key optimizations:
  - lintrans is purely row-wise, so it is computed only on the 4096 gathered
    rows AGE_inx[Endx] (4.8 GFLOP) instead of all 20000 rows (23.6 GFLOP).
  - All GEMMs run with bf16 operands (fp32 matmul is LOW_HIGH dual-pass on
    trn2, ~4-6x slower); accumulation stays fp32 in PSUM, all normalization /
    bias math stays fp32.
  - No collectives: o_c is column-sharded. Each core computes its own 512
    columns (from its local Endx shard) for ALL 8192 rows, streaming t in
    blocks. Avoids the AllGather's ~40-130us serial cost entirely.
  - Each core receives t with its 1024-row block rotated to the front, so the
    o_f row-shard stays a uniform SPMD program; the host un-rotates the o_c
    row blocks when reassembling.

Sharding over 8 cores:
  - E-branch sharded by Endx: core c gathers rows for Endx[512c:512c+512] and
    produces G^T shard [256, 512] (bf16) locally.
  - o_c column-sharded: core c outputs o_c[:, 512c:512c+512] (rows rotated).
  - o_f row-sharded: core c outputs o_f rows [1024c:1024c+1024].
"""

import numpy as np
import sys
from contextlib import ExitStack

for p in ("/opt/trn_rl_repo",):
    if p not in sys.path:
        sys.path.insert(0, p)

import concourse.bass as bass
import concourse.mybir as mybir
import concourse.tile as tile
from concourse import bass_utils
from concourse.masks import make_identity

F32 = mybir.dt.float32
BF16 = mybir.dt.bfloat16
F16 = mybir.dt.float16
I32 = mybir.dt.int32

NCORES = 8
N, NODES, DIN, HID, M, OUT_FE = 8192, 20000, 768, 256, 4096, 50
NI = N // NCORES          # 1024 rows per t-block
R = M // NCORES           # 512 gathered rows / o_c columns per core
P = 128
DTILES = DIN // P         # 6
JT = HID // P             # 2
RT = R // P               # 4


def build_program(tc: tile.TileContext, io: dict):
    """Emit the kernel IR. `io` maps logical names to DRAM APs."""
    nc = tc.nc

    tT = io["tT"]            # [768, 8192] bf16 (blocks rotated per core)
    AGE = io["AGE"]          # [20000, 768] bf16
    endx = io["endx"]        # [512] int32
    W_ageT = io["W_ageT"]    # [768, 768] bf16
    b_age = io["b_age"]      # [768] bf16
    W_prjTT = io["W_prjTT"]  # [768, 256] bf16
    b_prjT = io["b_prjT"]    # [256] f32
    W_prjLT = io["W_prjLT"]  # [768, 256] bf16
    b_prjL = io["b_prjL"]    # [256] f32
    outWT = io["outWT"]      # [256, 512] f32   this core's outW shard, transposed
    outb_s = io["outb_s"]    # [512] f32        this core's outb shard
    outb_h = io["outb_h"]    # [512] f16        same, fp16
    W_fc3T = io["W_fc3T"]    # [256, 50] bf16
    b_fc3 = io["b_fc3"]      # [50] bf16
    o_c = io["o_c"]          # [8192, 512] f32 out (this core's columns)
    o_f = io["o_f"]          # [1024, 50] f32 out (this core's rows)

    ctx_p = ExitStack()      # whole-kernel pools
    persist = ctx_p.enter_context(tc.tile_pool(name="persist", bufs=1))
    tpool = ctx_p.enter_context(tc.tile_pool(name="tpool", bufs=2))
    ocst = ctx_p.enter_context(tc.tile_pool(name="ocst", bufs=6))
    psum_mm = ctx_p.enter_context(tc.tile_pool(name="psum_mm", bufs=2, space="PSUM"))
    psum_oc = ctx_p.enter_context(tc.tile_pool(name="psum_oc", bufs=4, space="PSUM"))

    ctx_e = ExitStack()      # pools for the E branch
    arows = ctx_e.enter_context(tc.tile_pool(name="arows", bufs=2))
    zpool = ctx_e.enter_context(tc.tile_pool(name="zpool", bufs=2))
    sqpool = ctx_e.enter_context(tc.tile_pool(name="sqpool", bufs=2))
    stats = ctx_e.enter_context(tc.tile_pool(name="stats", bufs=8))
    psum_t = ctx_e.enter_context(tc.tile_pool(name="psum_t", bufs=2, space="PSUM"))

    # ---------------- constants / weights into SBUF ----------------
    ident_bf = persist.tile([P, P], BF16, tag="ident_bf")
    make_identity(nc, ident_bf)
    ident_f32 = persist.tile([P, P], F32, tag="ident_f32")
    make_identity(nc, ident_f32)
    ones_bf = persist.tile([1, P], BF16, tag="ones_bf")
    nc.vector.memset(ones_bf[:], 1.0)
    ones_h16 = persist.tile([1, P], F16, tag="ones_h16")
    nc.vector.memset(ones_h16[:], 1.0)

    idx_tiles = []
    for rt in range(RT):
        it_ = persist.tile([P, 1], I32, tag=f"idx_{rt}", name=f"idx_{rt}")
        nc.sync.dma_start(it_[:], endx[rt * P:(rt + 1) * P][:, None])
        idx_tiles.append(it_)

    b_age_row = persist.tile([1, DIN], BF16, tag="b_age_row")
    nc.sync.dma_start(b_age_row[:], b_age[None, :])
    bfc3_row = persist.tile([1, OUT_FE], BF16, tag="bfc3_row")
    nc.sync.dma_start(bfc3_row[:], b_fc3[None, :])
    outb_h_row = persist.tile([1, R], F16, tag="outb_h_row")
    nc.sync.dma_start(outb_h_row[:], outb_h[None, :])
    outb_bc = persist.tile([P, R], F32, tag="outb_bc")
    nc.sync.dma_start(outb_bc[:], outb_s[None, :].to_broadcast((P, R)))

    W_ageT_sb = []
    for d in range(DTILES):
        w = persist.tile([P, DIN], BF16, tag=f"W_ageT_{d}", name=f"W_ageT_{d}")
        nc.sync.dma_start(w[:], W_ageT[d * P:(d + 1) * P, :])
        W_ageT_sb.append(w)

    b_prjT_p = []
    b_prjL_p = []
    for j in range(JT):
        bt = persist.tile([P, 1], F32, tag=f"b_prjT_{j}", name=f"b_prjT_{j}")
        nc.sync.dma_start(bt[:], b_prjT[j * P:(j + 1) * P][:, None])
        b_prjT_p.append(bt)
        bl = persist.tile([P, 1], F32, tag=f"b_prjL_{j}", name=f"b_prjL_{j}")
        nc.sync.dma_start(bl[:], b_prjL[j * P:(j + 1) * P][:, None])
        b_prjL_p.append(bl)

    W_prjLT_sb = []
    W_prjTT_sb = []
    for d in range(DTILES):
        w = persist.tile([P, HID], BF16, tag=f"W_prjLT_{d}", name=f"W_prjLT_{d}")
        nc.sync.dma_start(w[:], W_prjLT[d * P:(d + 1) * P, :])
        W_prjLT_sb.append(w)
        w2 = persist.tile([P, HID], BF16, tag=f"W_prjTT_{d}", name=f"W_prjTT_{d}")
        nc.sync.dma_start(w2[:], W_prjTT[d * P:(d + 1) * P, :])
        W_prjTT_sb.append(w2)
    outWT_sb = []
    W_fc3T_sb = []
    for j in range(JT):
        w = persist.tile([P, R], F32, tag=f"outWT_{j}", name=f"outWT_{j}")
        nc.sync.dma_start(w[:], outWT[j * P:(j + 1) * P, :])
        outWT_sb.append(w)
        w2 = persist.tile([P, OUT_FE], BF16, tag=f"W_fc3T_{j}", name=f"W_fc3T_{j}")
        nc.sync.dma_start(w2[:], W_fc3T[j * P:(j + 1) * P, :])
        W_fc3T_sb.append(w2)

    # full h^T (bf16) [256, 8192], filled block by block
    hT = []
    for j in range(JT):
        h = persist.tile([P, N], BF16, tag=f"hT_{j}", name=f"hT_{j}")
        hT.append(h)

    def emit_h_block(ihb):
        """h^T for t-block ihb: hT[j][:, ihb*1024 : (ihb+1)*1024]."""
        tb = [tpool.tile([P, NI], BF16, tag=f"tb{d}", name=f"tb_{ihb}_{d}")
              for d in range(DTILES)]
        for d in range(DTILES):
            nc.sync.dma_start(tb[d][:], tT[d * P:(d + 1) * P,
                                           ihb * NI:(ihb + 1) * NI])
        for j in range(JT):
            for ih in range(NI // 512):
                hp = psum_mm.tile([P, 512], F32, tag="mm",
                                  name=f"hp_{ihb}_{j}_{ih}")
                for d in range(DTILES):
                    nc.tensor.matmul(hp[:], W_prjTT_sb[d][:, j * P:(j + 1) * P],
                                     tb[d][:, ih * 512:(ih + 1) * 512],
                                     start=(d == 0), stop=(d == DTILES - 1))
                nc.scalar.activation(
                    hT[j][:, ihb * NI + ih * 512:ihb * NI + (ih + 1) * 512],
                    hp[:], mybir.ActivationFunctionType.Tanh,
                    bias=b_prjT_p[j][:, :1], scale=1.0)

    # ---------------- E branch ----------------
    # gather rows (bf16) and transpose them: arT[d] = AGE_rows^T [128(d), 512(r)]
    arT = []
    for d in range(DTILES):
        a = persist.tile([P, R], BF16, tag=f"arT_{d}", name=f"arT_{d}")
        arT.append(a)
    for rt in range(RT):
        ar = arows.tile([P, DIN], BF16, tag="ar", name=f"ar_{rt}")
        nc.gpsimd.indirect_dma_start(
            out=ar[:],
            out_offset=None,
            in_=AGE[:],
            in_offset=bass.IndirectOffsetOnAxis(ap=idx_tiles[rt][:, :1], axis=0),
        )
        for d in range(DTILES):
            pt = psum_t.tile([P, P], BF16, tag="pt", name=f"ptA_{rt}_{d}")
            nc.tensor.transpose(pt[:], ar[:, d * P:(d + 1) * P], ident_bf[:])
            nc.any.tensor_copy(arT[d][:, rt * P:(rt + 1) * P], pt[:])

    # z = AGE_rows @ W_age^T + b_age (natural layout, fp32 accum), lintrans,
    # then transpose z_norm into znT[d] (bf16) [128(d), 512(r)]
    znT = []
    for d in range(DTILES):
        z = persist.tile([P, R], BF16, tag=f"znT_{d}", name=f"znT_{d}")
        znT.append(z)

    for rt in range(RT):
        z_sb = zpool.tile([P, DIN], F32, tag="z_sb", name=f"z_{rt}")
        for ch in range(2):  # two 384-wide output chunks
            zp = psum_mm.tile([P, 512], F32, tag="mm", name=f"zp_{rt}_{ch}")
            zps = zp[:, :384]
            nc.tensor.matmul(zps, ones_bf[:], b_age_row[:, ch * 384:(ch + 1) * 384],
                             start=True, stop=False)
            for d in range(DTILES):
                nc.tensor.matmul(
                    zps,
                    arT[d][:, rt * P:(rt + 1) * P],
                    W_ageT_sb[d][:, ch * 384:(ch + 1) * 384],
                    start=False, stop=(d == DTILES - 1),
                )
            nc.any.tensor_copy(z_sb[:, ch * 384:(ch + 1) * 384], zps)

        zmin = stats.tile([P, 1], F32, tag="zmin", name=f"zmin_{rt}")
        zmax = stats.tile([P, 1], F32, tag="zmax", name=f"zmax_{rt}")
        rng = stats.tile([P, 1], F32, tag="rng", name=f"rng_{rt}")
        rsc = stats.tile([P, 1], F32, tag="rsc", name=f"rsc_{rt}")
        ssq = stats.tile([P, 1], F32, tag="ssq", name=f"ssq_{rt}")
        nrm = stats.tile([P, 1], F32, tag="nrm", name=f"nrm_{rt}")
        rnm = stats.tile([P, 1], F32, tag="rnm", name=f"rnm_{rt}")
        nc.vector.tensor_reduce(zmin[:], z_sb[:], axis=mybir.AxisListType.X,
                                op=mybir.AluOpType.min)
        nc.vector.tensor_reduce(zmax[:], z_sb[:], axis=mybir.AxisListType.X,
                                op=mybir.AluOpType.max)
        nc.vector.tensor_tensor(rng[:], zmax[:], zmin[:],
                                op=mybir.AluOpType.subtract)
        nc.vector.reciprocal(rsc[:], rng[:])
        # z01 = (z - zmin) * rsc     (in place, fp32)
        nc.vector.tensor_scalar(z_sb[:], z_sb[:], zmin[:], rsc[:],
                                mybir.AluOpType.subtract, mybir.AluOpType.mult)
        sq = sqpool.tile([P, DIN], F32, tag="sq", name=f"sq_{rt}")
        nc.scalar.activation(sq[:], z_sb[:], mybir.ActivationFunctionType.Square,
                             accum_out=ssq[:])
        nc.scalar.sqrt(nrm[:], ssq[:])
        nc.vector.tensor_scalar_max(nrm[:], nrm[:], 1e-12)
        nc.vector.reciprocal(rnm[:], nrm[:])
        nc.vector.tensor_scalar_mul(z_sb[:], z_sb[:], rnm[:])
        # transpose z_norm (fp32 PE transpose), cast to bf16 on the copy out
        for d in range(DTILES):
            pt = psum_t.tile([P, P], F32, tag="pt", name=f"ptZ_{rt}_{d}")
            nc.tensor.transpose(pt[:], z_sb[:, d * P:(d + 1) * P], ident_f32[:])
            nc.any.tensor_copy(znT[d][:, rt * P:(rt + 1) * P], pt[:])

    # h block 0 here: fills the PE stall while lintrans stats run on DVE/ACT,
    # and provides the rows needed by o_f.
    emit_h_block(0)

    # E^T = tanh(W_prjL @ z_norm^T + b_prjL); G^T = E^T * outW^T  (bf16 out)
    gT = []
    for j in range(JT):
        ep = psum_mm.tile([P, 512], F32, tag="mm", name=f"ep_{j}")
        for d in range(DTILES):
            nc.tensor.matmul(ep[:], W_prjLT_sb[d][:, j * P:(j + 1) * P], znT[d][:],
                             start=(d == 0), stop=(d == DTILES - 1))
        eT = persist.tile([P, R], F32, tag=f"eT_{j}", name=f"eT_{j}")
        nc.scalar.activation(eT[:], ep[:], mybir.ActivationFunctionType.Tanh,
                             bias=b_prjL_p[j][:, :1], scale=1.0)
        g = persist.tile([P, R], BF16, tag=f"gT_{j}", name=f"gT_{j}")
        nc.vector.tensor_tensor(g[:], eT[:], outWT_sb[j][:],
                                op=mybir.AluOpType.mult)
        gT.append(g)

    ctx_e.close()

    # ---------------- o_f = h(block0) @ W_fc3^T + b_fc3 ----------------
    of_all = persist.tile([P, NI // P, OUT_FE], F32, tag="of_all")
    for it in range(NI // P):
        fp = psum_mm.tile([P, 512], F32, tag="mm", name=f"fp_{it}")
        fps = fp[:, :OUT_FE]
        nc.tensor.matmul(fps, ones_bf[:], bfc3_row[:], start=True, stop=False)
        for j in range(JT):
            nc.tensor.matmul(fps, hT[j][:, it * P:(it + 1) * P], W_fc3T_sb[j][:],
                             start=False, stop=(j == JT - 1))
        nc.any.tensor_copy(of_all[:, it, :], fps)
    nc.sync.dma_start(o_f.rearrange("(t p) f -> p t f", p=P), of_all[:])

    # ---------------- o_c[:, own 512 cols] = h @ gT + outb ----------------
    # Streams over the remaining h blocks; o_c tiles for block k overlap the
    # h GEMM of block k+1 on the PE.
    for ihb in range(N // NI):
        if ihb > 0:
            emit_h_block(ihb)
        for it in range(NI // P):
            g = ihb * (NI // P) + it
            op = psum_oc.tile([P, R], F32, tag="oc", name=f"op_{g}")
            dve_bias = (it % 2 == 0)
            if not dve_bias:
                nc.tensor.matmul(op[:], ones_h16[:], outb_h_row[:],
                                 start=True, stop=False)
            for j in range(JT):
                nc.tensor.matmul(op[:], hT[j][:, g * P:(g + 1) * P], gT[j][:],
                                 start=(j == 0 and dve_bias), stop=(j == JT - 1))
            stage = ocst.tile([P, R], F32, tag="ocs", name=f"ocs_{g}")
            if dve_bias:
                nc.vector.tensor_tensor(stage[:], op[:], outb_bc[:],
                                        op=mybir.AluOpType.add)
            else:
                nc.scalar.copy(stage[:], op[:])
            nc.sync.dma_start(o_c[g * P:(g + 1) * P, :], stage[:])

    ctx_p.close()


def legalize_sync(nc: bass.Bass):
    """The container's walrus accepts at most one sync wait and one sync
    update per instruction. Split extras onto adjacent NoOps (same engine,
    in-order queue => identical semantics)."""
    cnt = 0
    for func in nc.m.functions:
        for bb in func.blocks:
            new_insts = []
            for inst in bb.instructions:
                si = getattr(inst, "sync_info", None)
                waits = list(si.on_wait) if si is not None and si.on_wait else []
                if si is not None and len(waits) > 1:
                    for w in waits[:-1]:
                        cnt += 1
                        new_insts.append(mybir.InstNoOp(
                            name=f"{inst.name}_sw{cnt}",
                            sync_info=mybir.SyncInfo(on_wait=[w], on_update=[]),
                            bass_nofuse=True,
                            engine=inst.engine,
                        ))
                    si.on_wait = [waits[-1]]
                new_insts.append(inst)
                upds = list(si.on_update) if si is not None and si.on_update else []
                if si is not None and len(upds) > 1:
                    si.on_update = [upds[0]]
                    for u in upds[1:]:
                        cnt += 1
                        new_insts.append(mybir.InstNoOp(
                            name=f"{inst.name}_su{cnt}",
                            sync_info=mybir.SyncInfo(on_wait=[], on_update=[u]),
                            bass_nofuse=True,
                            engine=inst.engine,
                        ))
            bb.instructions[:] = new_insts
    return cnt


def build_bass() -> bass.Bass:
    nc = bass.Bass(trn_type="TRN2", num_devices=NCORES)
    io = {}
    io["tT"] = nc.dram_tensor("tT", [DIN, N], BF16, kind="ExternalInput").ap()
    io["AGE"] = nc.dram_tensor("AGE", [NODES, DIN], BF16, kind="ExternalInput").ap()
    io["endx"] = nc.dram_tensor("endx", [R], I32, kind="ExternalInput").ap()
    io["W_ageT"] = nc.dram_tensor("W_ageT", [DIN, DIN], BF16, kind="ExternalInput").ap()
    io["b_age"] = nc.dram_tensor("b_age", [DIN], BF16, kind="ExternalInput").ap()
    io["W_prjTT"] = nc.dram_tensor("W_prjTT", [DIN, HID], BF16, kind="ExternalInput").ap()
    io["b_prjT"] = nc.dram_tensor("b_prjT", [HID], F32, kind="ExternalInput").ap()
    io["W_prjLT"] = nc.dram_tensor("W_prjLT", [DIN, HID], BF16, kind="ExternalInput").ap()
    io["b_prjL"] = nc.dram_tensor("b_prjL", [HID], F32, kind="ExternalInput").ap()
    io["outWT"] = nc.dram_tensor("outWT", [HID, R], F32, kind="ExternalInput").ap()
    io["outb_s"] = nc.dram_tensor("outb_s", [R], F32, kind="ExternalInput").ap()
    io["outb_h"] = nc.dram_tensor("outb_h", [R], F16, kind="ExternalInput").ap()
    io["W_fc3T"] = nc.dram_tensor("W_fc3T", [HID, OUT_FE], BF16, kind="ExternalInput").ap()
    io["b_fc3"] = nc.dram_tensor("b_fc3", [OUT_FE], BF16, kind="ExternalInput").ap()
    io["o_c"] = nc.dram_tensor("o_c", [N, R], F32, kind="ExternalOutput").ap()
    io["o_f"] = nc.dram_tensor("o_f", [NI, OUT_FE], F32, kind="ExternalOutput").ap()
    with tile.TileContext(nc) as tc:
        build_program(tc, io)
    legalize_sync(nc)
    return nc


def make_in_maps(t, AGE_inx, Endx, W_age, b_age, W_prjT, b_prjT, W_prjL,
                 b_prjL, outW, outb, W_fc3, b_fc3):
    import ml_dtypes
    bf = lambda a: np.ascontiguousarray(np.asarray(a, np.float32).astype(ml_dtypes.bfloat16))
    f = lambda a: np.ascontiguousarray(np.asarray(a, np.float32))
    tT_full = np.asarray(t, np.float32).T.astype(ml_dtypes.bfloat16)  # [768, 8192]
    Endx = np.ascontiguousarray(np.asarray(Endx, np.int32))
    outW = np.asarray(outW, np.float32)
    outb = np.asarray(outb, np.float32)
    common = dict(
        AGE=bf(AGE_inx),
        W_ageT=bf(np.asarray(W_age, np.float32).T),
        b_age=bf(b_age),
        W_prjTT=bf(np.asarray(W_prjT, np.float32).T),
        b_prjT=f(b_prjT),
        W_prjLT=bf(np.asarray(W_prjL, np.float32).T),
        b_prjL=f(b_prjL),
        W_fc3T=bf(np.asarray(W_fc3, np.float32).T),
        b_fc3=bf(b_fc3),
    )
    in_maps = []
    for c in range(NCORES):
        # rotate the row blocks of t so block c is first
        order = [(c + k) % NCORES for k in range(NCORES)]
        tT_c = np.concatenate([tT_full[:, NI * b:NI * (b + 1)] for b in order],
                              axis=1)
        in_maps.append(dict(
            common,
            tT=np.ascontiguousarray(tT_c),
            endx=np.ascontiguousarray(Endx[R * c:R * (c + 1)]),
            outWT=f(outW[R * c:R * (c + 1)].T),
            outb_s=f(outb[R * c:R * (c + 1)]),
            outb_h=np.ascontiguousarray(
                outb[R * c:R * (c + 1)].astype(np.float16)),
        ))
    return in_maps


def assemble(results):
    """Un-rotate each core's o_c row blocks and concatenate columns; o_f rows
    concatenate in core order."""
    o_c = np.empty((N, M), np.float32)
    for c, r in enumerate(results):
        blk = r["o_c"]  # [8192, 512], row blocks rotated by c
        for k in range(NCORES):
            b = (c + k) % NCORES
            o_c[NI * b:NI * (b + 1), R * c:R * (c + 1)] = blk[NI * k:NI * (k + 1)]
    o_f = np.concatenate([r["o_f"] for r in results], axis=0)
    return o_c, o_f


_CACHED_NC = None


def kernel(**inputs):
    global _CACHED_NC
    if _CACHED_NC is None:
        _CACHED_NC = build_bass()
    in_maps = make_in_maps(**inputs)
    res = bass_utils.run_bass_kernel_spmd(
        _CACHED_NC, in_maps, core_ids=list(range(NCORES)))
    return assemble(res.results)


if __name__ == "__main__":
    nc = build_bass()
    print("build OK; instructions:",
          sum(len(bb.instructions) for bb in nc.main_func.blocks))
